# revision 10
# baseline (speedup 1.0000x reference)
"""Block-sparse linear kernel for Trainium2 (8 NeuronCores, SPMD).

y = W_blocksparse @ x + bias
  x:             [32768, 1024] f32   (128 in-blocks x 256)
  block_weights: [819, 256, 256] f32 (out x in per block)
  bias:          [16384, 1] f32      (64 out-blocks x 256)
  in_idx/out_idx:[819] int32
  y:             [16384, 1024] f32

Sharding: expert-style by out-block. The 64 out-blocks are partitioned into
8 groups of 8 (one per core, balanced by block count). Each core computes
its 8 out-blocks' rows of y over the full batch; outputs are disjoint, so
no collectives. Inputs are packed host-side into per-core fp16 arrays
(weights pre-transposed into lhsT tiles, x tiles pre-gathered per block);
the device program is uniform across cores (SPMD), with zero-weight padding
blocks equalizing per-position block counts.

Device compute: fp16 matmuls (1 cycle/row on the PE) accumulating in fp32
PSUM; bias added during the PSUM->SBUF eviction on the scalar engine.
"""

import functools
import hashlib
import os
import shutil

import numpy as np

NIB = 128      # input blocks
NOBT = 64      # total output blocks
BIN = 256
BOUT = 256
BATCH = 1024
NCORES = 8
NOB = NOBT // NCORES   # out-blocks per core
P = 128

_NEFF_CACHE = os.environ.get(
    "BASS_NEFF_CACHE", os.path.expanduser("~/.cache/bass_neff_cache")
)


def _install_neff_cache():
    """Disk-cache walrus NEFF compiles keyed on the BIR json hash."""
    import concourse.bass2jax as b2j

    if getattr(b2j, "_neff_disk_cache_installed", False):
        return
    orig = b2j.compile_bir_kernel

    def cached(bir_json, tmpdir, neff_name="file.neff"):
        data = bir_json if isinstance(bir_json, bytes) else bir_json.encode()
        key = hashlib.sha256(data).hexdigest()
        cpath = os.path.join(_NEFF_CACHE, key + ".neff")
        if os.path.exists(cpath):
            dst = os.path.join(tmpdir, neff_name)
            shutil.copy(cpath, dst)
            return dst
        out = orig(bir_json, tmpdir, neff_name=neff_name)
        try:
            os.makedirs(_NEFF_CACHE, exist_ok=True)
            tmp = cpath + ".tmp%d" % os.getpid()
            shutil.copy(out, tmp)
            os.replace(tmp, cpath)
        except OSError:
            pass
        return out

    b2j.compile_bir_kernel = cached
    b2j._neff_disk_cache_installed = True


def _plan(in_idx, out_idx):
    """Partition the 64 out-blocks into 8 balanced groups of 8 and compute
    the (cross-core shared) padded per-position block counts.

    The device program is identical on all cores (SPMD), so position pos of
    every core must process the same padded count c[pos] = max over cores of
    the out-block size at that position. Two heuristics (greedy bin-pack and
    snake deal) are tried; the one with less padding wins."""
    counts = np.bincount(out_idx, minlength=NOBT)
    order = np.argsort(-counts, kind="stable")

    def padded(groups):
        for g in range(NCORES):
            groups[g].sort(key=lambda ob: (-counts[ob], ob))
        c = tuple(
            max(1, max(int(counts[groups[g][pos]]) for g in range(NCORES)))
            for pos in range(NOB)
        )
        return groups, c

    # greedy: largest first onto the lightest non-full group
    greedy = [[] for _ in range(NCORES)]
    tot = [0] * NCORES
    for ob in order:
        cands = [g for g in range(NCORES) if len(greedy[g]) < NOB]
        g = min(cands, key=lambda gg: tot[gg])
        greedy[g].append(int(ob))
        tot[g] += int(counts[ob])
    # snake deal: rank r goes to group snake(r)
    snake = [[] for _ in range(NCORES)]
    for r, ob in enumerate(order):
        k = r % (2 * NCORES)
        g = k if k < NCORES else 2 * NCORES - 1 - k
        snake[g].append(int(ob))

    best = min((padded(gr) for gr in (greedy, snake)),
               key=lambda gc: sum(gc[1]))
    groups, c = best
    blocks_by_ob = [np.nonzero(out_idx == ob)[0] for ob in range(NOBT)]
    return groups, c, blocks_by_ob


@functools.lru_cache(maxsize=8)
def _build_program(c, iters=1):
    """Build + compile the uniform SPMD Tile program for padded counts c.

    iters > 1 wraps the whole body in an on-device For_i loop repeating the
    identical computation — used only for timing (amortizes dispatch RPC
    overhead into a measurable on-device duration).
    """
    import contextlib

    from concourse import bacc, mybir, tile

    f16 = mybir.dt.float16
    f32 = mybir.dt.float32
    NB = sum(c)

    nc = bacc.Bacc("TRN2", target_bir_lowering=False, debug=False,
                   num_devices=NCORES)
    # per block: [lhsT tiles (512) | x k-tile 0 (1024) | x k-tile 1 (1024)]
    # packed p-major so each block is one contiguous 640 KB DMA.
    wx_ext = nc.dram_tensor("wx", [NB, P, 2560], f16,
                            kind="ExternalInput").ap()
    b_ext = nc.dram_tensor("bias", [P, 2 * NOB], f32,
                           kind="ExternalInput").ap()
    y_ext = nc.dram_tensor("y", [NOB * BOUT, BATCH], f32,
                           kind="ExternalOutput").ap()

    with tile.TileContext(nc) as tc:
        with tc.tile_pool(name="wxp", bufs=14) as wxp, \
             tc.tile_pool(name="yp", bufs=6) as yp, \
             tc.tile_pool(name="bp", bufs=1) as bp, \
             tc.tile_pool(name="psp", bufs=8, space="PSUM") as psp:
            bt = bp.tile([P, 2 * NOB], f32, tag="bias", name="bt")
            nc.sync.dma_start(out=bt[:], in_=b_ext[:])
            loop = (
                tc.For_i(0, iters, 1,
                         hint_engines=(mybir.EngineType.PE,
                                       mybir.EngineType.SP,
                                       mybir.EngineType.DVE))
                if iters > 1 else contextlib.nullcontext()
            )
            with loop:
                _emit_body(nc, tc, c, wx_ext, y_ext, bt, wxp, yp, psp,
                           f16, f32)
    nc.compile()
    return nc


def _emit_body(nc, tc, c, wx_ext, y_ext, bt, wxp, yp, psp, f16, f32):
    j0 = 0
    for g in range(NOB):
        ps = [psp.tile([P, 512], f32, tag="ps", name="ps") for _ in range(4)]
        for jj in range(c[g]):
            j = j0 + jj
            wxt = wxp.tile([P, 2560], f16, tag="wx", name="wxt")
            nc.sync.dma_start(out=wxt[:], in_=wx_ext[j])
            for kt in range(2):
                for mt in range(2):
                    lhs = wxt[:, (kt * 2 + mt) * P:(kt * 2 + mt + 1) * P]
                    for nn in range(2):
                        nc.tensor.matmul(
                            ps[mt * 2 + nn][:],
                            lhsT=lhs,
                            rhs=wxt[:, 512 + kt * BATCH + nn * 512:
                                    512 + kt * BATCH + (nn + 1) * 512],
                            start=(jj == 0 and kt == 0),
                            stop=(jj == c[g] - 1 and kt == 1),
                        )
        j0 += c[g]
        for mt in range(2):
            yt = yp.tile([P, BATCH], f32, tag="y", name="yt")
            for nn in range(2):
                nc.vector.tensor_scalar_add(
                    out=yt[:, nn * 512:(nn + 1) * 512],
                    in0=ps[mt * 2 + nn][:],
                    scalar1=bt[:, g * 2 + mt:g * 2 + mt + 1],
                )
            row = (g * 2 + mt) * P
            nc.sync.dma_start(out=y_ext[row:row + P, :], in_=yt[:])


def _pack_inputs(x, block_weights, bias, in_idx, groups, c, blocks_by_ob):
    """Host-side packing into per-core fp16 input arrays."""
    NB = sum(c)
    # lhsT tiles: wpack[n, p, kt, mt, cc] = W[n].T[kt*128+p, mt*128+cc]
    wpack = np.ascontiguousarray(
        block_weights.transpose(0, 2, 1)
        .reshape(-1, 2, P, 2, P)
        .transpose(0, 2, 1, 3, 4)
    ).astype(np.float16).reshape(-1, P, 512)
    # x tiles p-major: xcomb[ib, p, kt*1024 + b]
    xcomb = np.ascontiguousarray(
        x.astype(np.float16).reshape(NIB, 2, P, BATCH).transpose(0, 2, 1, 3)
    ).reshape(NIB, P, 2 * BATCH)

    in_maps = []
    for g in range(NCORES):
        wx_core = np.zeros((NB, P, 2560), np.float16)
        bias_core = np.zeros((P, 2 * NOB), np.float32)
        j0 = 0
        for pos in range(NOB):
            ob = groups[g][pos]
            blocks = blocks_by_ob[ob]
            nblk = len(blocks)
            if nblk:
                wx_core[j0:j0 + nblk, :, :512] = wpack[blocks]
                wx_core[j0:j0 + nblk, :, 512:] = xcomb[in_idx[blocks]]
            for mt in range(2):
                bias_core[:, pos * 2 + mt] = bias[ob * BOUT + mt * P:
                                                  ob * BOUT + (mt + 1) * P, 0]
            j0 += c[pos]
        in_maps.append({"wx": wx_core, "bias": bias_core})
    return in_maps


# Exposed for the test harness: last-built program + inputs for re-timing.
_last = {}


def kernel(x, block_weights, bias, in_idx, out_idx):
    _install_neff_cache()
    from concourse.bass_utils import run_bass_kernel_spmd

    x = np.asarray(x, dtype=np.float32)
    block_weights = np.asarray(block_weights, dtype=np.float32)
    bias = np.asarray(bias, dtype=np.float32)
    in_idx = np.asarray(in_idx, dtype=np.int64)
    out_idx = np.asarray(out_idx, dtype=np.int64)

    groups, c, blocks_by_ob = _plan(in_idx, out_idx)
    nc = _build_program(c)
    in_maps = _pack_inputs(x, block_weights, bias, in_idx, groups, c,
                           blocks_by_ob)

    res = run_bass_kernel_spmd(nc, in_maps, core_ids=list(range(NCORES)))

    y = np.empty((NOBT * BOUT, BATCH), np.float32)
    for g in range(NCORES):
        yc = res.results[g]["y"]
        for pos in range(NOB):
            ob = groups[g][pos]
            y[ob * BOUT:(ob + 1) * BOUT, :] = yc[pos * BOUT:(pos + 1) * BOUT, :]

    _last.update(nc=nc, in_maps=in_maps, groups=groups, c=c)
    return y


# revision 24
# speedup vs baseline: 1.2348x; 1.2348x over previous
"""Block-sparse linear kernel for Trainium2 (8 NeuronCores, SPMD).

y = W_blocksparse @ x + bias
  x:             [32768, 1024] f32   (128 in-blocks x 256)
  block_weights: [819, 256, 256] f32 (out x in per block)
  bias:          [16384, 1] f32      (64 out-blocks x 256)
  in_idx/out_idx:[819] int32
  y:             [16384, 1024] f32

Sharding: expert-style by out-block. The 64 out-blocks are partitioned into
8 groups of 8 (one per core, balanced by block count). Each core computes
its 8 out-blocks' rows of y over the full batch; outputs are disjoint, so
no collectives. Inputs are packed host-side into per-core fp16 arrays
(weights pre-transposed into lhsT tiles, x tiles pre-gathered per block);
the device program is uniform across cores (SPMD), with zero-weight padding
blocks equalizing per-position block counts.

Device compute: fp16 matmuls (1 cycle/row on the PE) accumulating in fp32
PSUM; bias added during the PSUM->SBUF eviction on the scalar engine.
"""

import functools
import hashlib
import os
import shutil

import numpy as np

NIB = 128      # input blocks
NOBT = 64      # total output blocks
BIN = 256
BOUT = 256
BATCH = 1024
NCORES = 8
NOB = NOBT // NCORES   # out-blocks per core
P = 128

_NEFF_CACHE = os.environ.get(
    "BASS_NEFF_CACHE", os.path.expanduser("~/.cache/bass_neff_cache")
)


def _install_neff_cache():
    """Disk-cache walrus NEFF compiles keyed on the BIR json hash."""
    import concourse.bass2jax as b2j

    if getattr(b2j, "_neff_disk_cache_installed", False):
        return
    orig = b2j.compile_bir_kernel

    def cached(bir_json, tmpdir, neff_name="file.neff"):
        data = bir_json if isinstance(bir_json, bytes) else bir_json.encode()
        key = hashlib.sha256(data).hexdigest()
        cpath = os.path.join(_NEFF_CACHE, key + ".neff")
        if os.path.exists(cpath):
            dst = os.path.join(tmpdir, neff_name)
            shutil.copy(cpath, dst)
            return dst
        out = orig(bir_json, tmpdir, neff_name=neff_name)
        try:
            os.makedirs(_NEFF_CACHE, exist_ok=True)
            tmp = cpath + ".tmp%d" % os.getpid()
            shutil.copy(out, tmp)
            os.replace(tmp, cpath)
        except OSError:
            pass
        return out

    b2j.compile_bir_kernel = cached
    b2j._neff_disk_cache_installed = True


def _plan(in_idx, out_idx):
    """Partition the 64 out-blocks into 8 balanced groups of 8 and compute
    the (cross-core shared) padded per-position block counts.

    The device program is identical on all cores (SPMD), so position pos of
    every core must process the same padded count c[pos] = max over cores of
    the out-block size at that position. Two heuristics (greedy bin-pack and
    snake deal) are tried; the one with less padding wins."""
    counts = np.bincount(out_idx, minlength=NOBT)
    order = np.argsort(-counts, kind="stable")

    def padded(groups):
        for g in range(NCORES):
            groups[g].sort(key=lambda ob: (-counts[ob], ob))
        c = tuple(
            max(1, max(int(counts[groups[g][pos]]) for g in range(NCORES)))
            for pos in range(NOB)
        )
        return groups, c

    # greedy: largest first onto the lightest non-full group
    greedy = [[] for _ in range(NCORES)]
    tot = [0] * NCORES
    for ob in order:
        cands = [g for g in range(NCORES) if len(greedy[g]) < NOB]
        g = min(cands, key=lambda gg: tot[gg])
        greedy[g].append(int(ob))
        tot[g] += int(counts[ob])
    # snake deal: rank r goes to group snake(r)
    snake = [[] for _ in range(NCORES)]
    for r, ob in enumerate(order):
        k = r % (2 * NCORES)
        g = k if k < NCORES else 2 * NCORES - 1 - k
        snake[g].append(int(ob))

    best = min((padded(gr) for gr in (greedy, snake)),
               key=lambda gc: sum(gc[1]))
    groups, c = best
    blocks_by_ob = [np.nonzero(out_idx == ob)[0] for ob in range(NOBT)]
    return groups, c, blocks_by_ob


# DMA tuning knobs: width (f16 columns) of each dense x DMA piece, issuing
# engines (round-robin), and wx pool depth.
_DEF_XW = 2048
_DEF_ENGINES = ("sync", "scalar")
_DEF_BUFS = 14


@functools.lru_cache(maxsize=32)
def _build_program(c, iters=1, xw=_DEF_XW, engines=_DEF_ENGINES,
                   bufs=_DEF_BUFS, mode="full"):
    """Build + compile the uniform SPMD Tile program for padded counts c.

    iters > 1 wraps the whole body in an on-device For_i loop repeating the
    identical computation — used only for timing (amortizes dispatch RPC
    overhead into a measurable on-device duration).
    """
    import contextlib

    from concourse import bacc, mybir, tile

    f16 = mybir.dt.float16
    f32 = mybir.dt.float32
    NB = sum(c)

    npx = (2 * BATCH) // xw     # dense x pieces per block
    nc = bacc.Bacc("TRN2", target_bir_lowering=False, debug=False,
                   num_devices=NCORES)
    w_ext = nc.dram_tensor("w", [NB, P, 512], f16, kind="ExternalInput").ap()
    xs_ext = nc.dram_tensor("xs", [NB, npx, P, xw], f16,
                            kind="ExternalInput").ap()
    b_ext = nc.dram_tensor("bias", [P, 2 * NOB], f32,
                           kind="ExternalInput").ap()
    y_ext = nc.dram_tensor("y", [NOB * BOUT, BATCH], f32,
                           kind="ExternalOutput").ap()

    with tile.TileContext(nc) as tc:
        with tc.tile_pool(name="wp", bufs=bufs) as wp, \
             tc.tile_pool(name="xp", bufs=bufs) as xp, \
             tc.tile_pool(name="yp", bufs=6) as yp, \
             tc.tile_pool(name="bp", bufs=1) as bp, \
             tc.tile_pool(name="psp", bufs=8, space="PSUM") as psp:
            bt = bp.tile([P, 2 * NOB], f32, tag="bias", name="bt")
            nc.sync.dma_start(out=bt[:], in_=b_ext[:])
            loop = (
                tc.For_i(0, iters, 1,
                         hint_engines=(mybir.EngineType.PE,
                                       mybir.EngineType.SP,
                                       mybir.EngineType.DVE))
                if iters > 1 else contextlib.nullcontext()
            )
            with loop:
                _emit_body(nc, tc, c, w_ext, xs_ext, y_ext, bt, wp, xp,
                           yp, psp, f16, f32, xw, engines, mode)
    nc.compile()
    return nc


def _emit_body(nc, tc, c, w_ext, xs_ext, y_ext, bt, wp, xp, yp, psp,
               f16, f32, xw, engines, mode="full"):
    npx = (2 * BATCH) // xw
    eng_rr = [getattr(nc, e) for e in engines]
    do_dma = mode in ("full", "dma", "full_ldw")
    do_pe = mode in ("full", "pe", "pe1", "pe_ldw", "full_ldw")
    if mode in ("pe", "pe1", "pe_ldw"):
        # static operand tiles loaded once; matmul stream only
        wt_s = wp.tile([P, 512], f16, tag="w", name="wt_s", bufs=1)
        nc.sync.dma_start(out=wt_s[:], in_=w_ext[0])
        xt_s = xp.tile([P, 2 * BATCH], f16, tag="x", name="xt_s", bufs=1)
        for i in range(npx):
            nc.sync.dma_start(out=xt_s[:, i * xw:(i + 1) * xw],
                              in_=xs_ext[0, i])
    n_dma = 0
    j0 = 0
    for g in range(NOB):
        ps = [psp.tile([P, 512], f32, tag="ps", name="ps") for _ in range(4)]
        for jj in range(c[g]):
            j = j0 + jj
            if do_dma:
                wt = wp.tile([P, 512], f16, tag="w", name="wt")
                eng = eng_rr[n_dma % len(eng_rr)]
                n_dma += 1
                eng.dma_start(out=wt[:], in_=w_ext[j])
                xt = xp.tile([P, 2 * BATCH], f16, tag="x", name="xt")
                for i in range(npx):
                    eng = eng_rr[n_dma % len(eng_rr)]
                    n_dma += 1
                    eng.dma_start(out=xt[:, i * xw:(i + 1) * xw],
                                  in_=xs_ext[j, i])
            else:
                wt, xt = wt_s, xt_s
            if do_pe:
                for kt in range(2):
                    for mt in range(2):
                        if mode == "pe1":
                            lhs = wt[:, 0:P]
                        else:
                            lhs = wt[:, (kt * 2 + mt) * P:(kt * 2 + mt + 1) * P]
                        if mode in ("pe_ldw", "full_ldw"):
                            nc.tensor.ldweights(lhs)
                        for nn in range(2):
                            nc.tensor.matmul(
                                ps[mt * 2 + nn][:],
                                lhsT=lhs,
                                rhs=xt[:, kt * BATCH + nn * 512:
                                       kt * BATCH + (nn + 1) * 512],
                                start=(jj == 0 and kt == 0),
                                stop=(jj == c[g] - 1 and kt == 1),
                            )
        j0 += c[g]
        for mt in range(2):
            yt = yp.tile([P, BATCH], f32, tag="y", name="yt")
            if do_pe:
                for nn in range(2):
                    nc.vector.tensor_scalar_add(
                        out=yt[:, nn * 512:(nn + 1) * 512],
                        in0=ps[mt * 2 + nn][:],
                        scalar1=bt[:, g * 2 + mt:g * 2 + mt + 1],
                    )
            else:
                # dma mode: make the x/w tiles observable so nothing is
                # dead-code-eliminated — cast-copy a sliver into yt
                nc.vector.tensor_copy(out=yt[:, :512], in_=xt[:, :512])
                nc.vector.tensor_copy(out=yt[:, 512:], in_=wt[:, :512])
            row = (g * 2 + mt) * P
            eng = eng_rr[n_dma % len(eng_rr)]
            n_dma += 1
            eng.dma_start(out=y_ext[row:row + P, :], in_=yt[:])


def _pack_inputs(x, block_weights, bias, in_idx, groups, c, blocks_by_ob,
                 xw=_DEF_XW):
    """Host-side packing into per-core fp16 input arrays."""
    NB = sum(c)
    npx = (2 * BATCH) // xw
    # lhsT tiles: wpack[n, p, kt, mt, cc] = W[n].T[kt*128+p, mt*128+cc]
    wpack = np.ascontiguousarray(
        block_weights.transpose(0, 2, 1)
        .reshape(-1, 2, P, 2, P)
        .transpose(0, 2, 1, 3, 4)
    ).astype(np.float16).reshape(-1, P, 512)
    # x pieces, each dense [P, xw]: piece i of block = columns
    # [i*xw, (i+1)*xw) of the p-major [P, 2048] per-in-block matrix
    xcomb = np.ascontiguousarray(
        x.astype(np.float16).reshape(NIB, 2, P, BATCH).transpose(0, 2, 1, 3)
        .reshape(NIB, P, 2 * BATCH)
        .reshape(NIB, P, npx, xw)
        .transpose(0, 2, 1, 3)
    )  # [NIB, npx, P, xw]

    in_maps = []
    for g in range(NCORES):
        w_core = np.zeros((NB, P, 512), np.float16)
        xs_core = np.zeros((NB, npx, P, xw), np.float16)
        bias_core = np.zeros((P, 2 * NOB), np.float32)
        j0 = 0
        for pos in range(NOB):
            ob = groups[g][pos]
            blocks = blocks_by_ob[ob]
            nblk = len(blocks)
            if nblk:
                w_core[j0:j0 + nblk] = wpack[blocks]
                xs_core[j0:j0 + nblk] = xcomb[in_idx[blocks]]
            for mt in range(2):
                bias_core[:, pos * 2 + mt] = bias[ob * BOUT + mt * P:
                                                  ob * BOUT + (mt + 1) * P, 0]
            j0 += c[pos]
        in_maps.append({"w": w_core, "xs": xs_core, "bias": bias_core})
    return in_maps


# Exposed for the test harness: last-built program + inputs for re-timing.
_last = {}


def kernel(x, block_weights, bias, in_idx, out_idx):
    _install_neff_cache()
    from concourse.bass_utils import run_bass_kernel_spmd

    x = np.asarray(x, dtype=np.float32)
    block_weights = np.asarray(block_weights, dtype=np.float32)
    bias = np.asarray(bias, dtype=np.float32)
    in_idx = np.asarray(in_idx, dtype=np.int64)
    out_idx = np.asarray(out_idx, dtype=np.int64)

    groups, c, blocks_by_ob = _plan(in_idx, out_idx)
    nc = _build_program(c)
    in_maps = _pack_inputs(x, block_weights, bias, in_idx, groups, c,
                           blocks_by_ob)

    res = run_bass_kernel_spmd(nc, in_maps, core_ids=list(range(NCORES)))

    y = np.empty((NOBT * BOUT, BATCH), np.float32)
    for g in range(NCORES):
        yc = res.results[g]["y"]
        for pos in range(NOB):
            ob = groups[g][pos]
            y[ob * BOUT:(ob + 1) * BOUT, :] = yc[pos * BOUT:(pos + 1) * BOUT, :]

    _last.update(nc=nc, in_maps=in_maps, groups=groups, c=c)
    return y
